# revision 39
# baseline (speedup 1.0000x reference)
"""NMS detection-metric (greedy matching mean-precision) on 8 Trainium2 cores.

Data-parallel over images (16 images/core), two device launches.

Launch 1 (candidate detection, fp16 with conservative margin):
  pred-broadcast layout [gt-tile, pred] = [128|72, 2000].  Host ships pred
  planes (px1,py1,px2,py2,parea/3) pre-transposed + fp16, broadcast once
  per image as one DMA; gt side rides per-partition fp32 scalars.  The
  iou>=0.5 test is division-free (3*inter >= garea+parea-eps) with a
  host-computed per-gt margin eps_g that provably covers all fp16
  rounding, so the candidate set is a superset of the exact one (extra
  candidates are harmless: phase B recomputes exact fp32 iou and the
  scan rejects them).  Per-pred indicators come from a bf16 ones-column
  PE matmul into one [IPC,2048] PSUM tile; a single scan gives each
  candidate its dense slot.  Output: cumind = cumsum*indicator (fp16,
  exact for counts <= 2048) + per-image counts.

Host: picks smallest compiled K bucket >= max count + 8.

Launch 2 (K-specialized):
  Phase A: oh[r,slot] = (cumind[r] == slot) via fp16 tensor_scalar
           (4x mode); candidate boxes gathered by bf16 PE matmuls with
           the coordinates split hi/lo1/lo2 across three bf16 planes
           (exact to ~2e-5 px).
  Phase B: exact fp32 iou rows [128=(s,i), 200] per k-group written
           k-grouped to DRAM scratch.
  Phase C: greedy scan, 2 DVE ops per candidate slot:
             TTR: masked = row - f,  vt = max(max_col masked, thr)
             STT: f' = (masked >= vt) | f
           over [80,200] (16 images x 5 thresholds on partitions).
           Rows arrive 8 steps per DMA from the k-grouped scratch.
  tp = sum(f); host epilogue: precision = tp/(N+M-tp), mean.
"""

import numpy as np
from contextlib import ExitStack

B, N, M = 128, 2000, 200
NCORES = 8
IPC = B // NCORES            # images per core
NT = 5                       # thresholds
CH = IPC * NT                # chains per core (80)
NRT = (N + 127) // 128       # 16 pred row-tiles per image
RPAD = NRT * 128             # 2048
GT_TILES = ((0, 128), (128, M - 128))
K_BUCKETS = tuple(range(160, 2017, 32)) + (2048,)
THRESHOLDS = np.asarray(np.arange(0.5, 0.75, 0.05), np.float32)

_CACHE = {}


def _slot_chunks(K):
    out = []
    s = 0
    while s < K:
        out.append((s, min(128, K - s)))
        s += 128
    return out


def _build_p1():
    """Launch 1: fp16 margin candidate detection + slot assignment."""
    import concourse.tile as tile
    from concourse import bacc, mybir

    f32 = mybir.dt.float32
    f16 = mybir.dt.float16
    bf16 = mybir.dt.bfloat16
    OP = mybir.AluOpType
    AF = mybir.ActivationFunctionType

    nc = bacc.Bacc("TRN2", target_bir_lowering=False, debug=False,
                   num_devices=NCORES)

    # pbh planes per image: px1, py1, px2, py2, parea/3  (fp16, padded)
    pbh_d = nc.dram_tensor("pbh", [IPC, 5, RPAD], f16, kind="ExternalInput").ap()
    # gsc[p, (i*2+tile)*5 + c]: gx1, gy1, gx2, gy2, garea3p (fp32)
    gsc_d = nc.dram_tensor("gsc", [128, IPC * 2 * 5], f32,
                           kind="ExternalInput").ap()
    # colsel[p, i*IPC + j] = (j == i): routes image i's count to PSUM row i
    colsel_d = nc.dram_tensor("colsel", [128, IPC * IPC], bf16,
                              kind="ExternalInput").ap()
    cumind_o = nc.dram_tensor("cumind", [IPC, RPAD], f16,
                              kind="ExternalOutput").ap()
    cc_o = nc.dram_tensor("cc", [IPC, 1], f16, kind="ExternalOutput").ap()

    with tile.TileContext(nc) as tc, ExitStack() as ctx:
        cpool = ctx.enter_context(tc.tile_pool(name="const", bufs=1))
        colsel = cpool.tile([128, IPC * IPC], bf16)
        nc.sync.dma_start(colsel[:], colsel_d[:, :])
        gsc = cpool.tile([128, IPC * 2 * 5], f32)
        nc.sync.dma_start(gsc[:], gsc_d[:, :])

        with (
            tc.tile_pool(name="pb", bufs=2) as ppool,
            tc.tile_pool(name="work", bufs=2) as wpool,
            tc.tile_pool(name="cnt", bufs=1, space="PSUM") as cntpool,
        ):
            cnt = cntpool.tile([IPC, RPAD], f32)
            for i in range(IPC):
                pb = ppool.tile([128, 5 * RPAD], f16, tag="pb", name="pb")
                nc.sync.dma_start(
                    pb[:],
                    pbh_d[i].unsqueeze(0).to_broadcast([128, 5, RPAD]),
                )
                px1 = pb[:, 0 * RPAD : 1 * RPAD]
                py1 = pb[:, 1 * RPAD : 2 * RPAD]
                px2 = pb[:, 2 * RPAD : 3 * RPAD]
                py2 = pb[:, 3 * RPAD : 4 * RPAD]
                pa3 = pb[:, 4 * RPAD : 5 * RPAD]
                for tix, (off, rows) in enumerate(GT_TILES):
                    sc = (i * 2 + tix) * 5
                    gx1 = gsc[:rows, sc + 0 : sc + 1]
                    gy1 = gsc[:rows, sc + 1 : sc + 2]
                    gx2 = gsc[:rows, sc + 2 : sc + 3]
                    gy2 = gsc[:rows, sc + 3 : sc + 4]
                    ga3 = gsc[:rows, sc + 4 : sc + 5]
                    m2x = wpool.tile([128, RPAD], f16, tag="m2x", name="m2x")
                    m1x = wpool.tile([128, RPAD], f16, tag="m1x", name="m1x")
                    m2y = wpool.tile([128, RPAD], f16, tag="m2y", name="m2y")
                    m1y = wpool.tile([128, RPAD], f16, tag="m1y", name="m1y")
                    wx = wpool.tile([128, RPAD], f16, tag="wx", name="wx")
                    wy = wpool.tile([128, RPAD], f16, tag="wy", name="wy")
                    rwx = wpool.tile([128, RPAD], f16, tag="rwx", name="rwx")
                    rwy = wpool.tile([128, RPAD], f16, tag="rwy", name="rwy")
                    inter = wpool.tile([128, RPAD], f16, tag="inter",
                                       name="inter")
                    gp = wpool.tile([128, RPAD], f16, tag="gp", name="gp")
                    cmp = wpool.tile([128, RPAD], bf16, tag="cmp", name="cmp")
                    # STT never reaches 2x mode; ts+TT do at fp16.
                    nc.vector.tensor_scalar(
                        out=m2x[:rows], in0=px2[:rows], scalar1=gx2,
                        scalar2=None, op0=OP.min,
                    )
                    nc.vector.tensor_scalar(
                        out=m1x[:rows], in0=px1[:rows], scalar1=gx1,
                        scalar2=None, op0=OP.max,
                    )
                    nc.vector.tensor_tensor(
                        out=wx[:rows], in0=m2x[:rows], in1=m1x[:rows],
                        op=OP.subtract,
                    )
                    nc.scalar.activation(
                        out=rwx[:rows], in_=wx[:rows], func=AF.Relu, scale=3.0
                    )
                    nc.vector.tensor_scalar(
                        out=m2y[:rows], in0=py2[:rows], scalar1=gy2,
                        scalar2=None, op0=OP.min,
                    )
                    nc.vector.tensor_scalar(
                        out=m1y[:rows], in0=py1[:rows], scalar1=gy1,
                        scalar2=None, op0=OP.max,
                    )
                    nc.vector.tensor_tensor(
                        out=wy[:rows], in0=m2y[:rows], in1=m1y[:rows],
                        op=OP.subtract,
                    )
                    nc.scalar.activation(
                        out=rwy[:rows], in_=wy[:rows], func=AF.Relu, scale=1.0
                    )
                    # inter = 3*wx*wy
                    nc.vector.tensor_tensor(
                        out=inter[:rows], in0=rwx[:rows], in1=rwy[:rows],
                        op=OP.mult,
                    )
                    # candidate test: 3*inter >= parea + (garea - eps)
                    nc.vector.tensor_scalar(
                        out=gp[:rows], in0=pa3[:rows], scalar1=ga3,
                        scalar2=None, op0=OP.add,
                    )
                    nc.vector.tensor_tensor(
                        out=cmp[:rows], in0=inter[:rows], in1=gp[:rows],
                        op=OP.is_ge,
                    )
                    # PSUM bank = 512 fp32: chunk the count matmul.
                    # colsel block i accumulates into PSUM row i; the
                    # [IPC, RPAD] tile accumulates across all (i, tix).
                    for ch in range(4):
                        nc.tensor.matmul(
                            out=cnt[:, ch * 512 : (ch + 1) * 512],
                            lhsT=colsel[:rows, i * IPC : (i + 1) * IPC],
                            rhs=cmp[:rows, ch * 512 : (ch + 1) * 512],
                            start=(i == 0 and tix == 0),
                            stop=(i == IPC - 1 and tix == len(GT_TILES) - 1),
                        )

            with tc.tile_pool(name="tail", bufs=1) as spool:
                indf = spool.tile([IPC, RPAD], f16)
                indb = spool.tile([IPC, RPAD], f16)
                cum = spool.tile([IPC, RPAD], f16)
                cmi = spool.tile([IPC, RPAD], f16)
                nc.scalar.copy(out=indf[:], in_=cnt[:])
                nc.vector.tensor_scalar(
                    out=indb[:], in0=indf[:], scalar1=0.5, scalar2=None,
                    op0=OP.is_ge,
                )
                nc.vector.tensor_tensor_scan(
                    out=cum[:], data0=indb[:], data1=indb[:], initial=0.0,
                    op0=OP.add, op1=OP.bypass,
                )
                nc.vector.tensor_tensor(
                    out=cmi[:], in0=cum[:], in1=indb[:], op=OP.mult
                )
                nc.sync.dma_start(cumind_o[:, :], cmi[:])
                nc.sync.dma_start(cc_o[:, :], cum[:, RPAD - 1 : RPAD])

    nc.compile()
    return nc


def _build_p2(K):
    """Launch 2: gather + exact iou + 2-op greedy scan, specialized for K."""
    import concourse.tile as tile
    from concourse import bacc, mybir

    f32 = mybir.dt.float32
    f16 = mybir.dt.float16
    bf16 = mybir.dt.bfloat16
    OP = mybir.AluOpType
    AF = mybir.ActivationFunctionType
    AX = mybir.AxisListType

    NQ = (IPC * K) // 128        # k-groups of 8 slots x 16 images
    chunks = _slot_chunks(K)
    NCH = len(chunks)

    nc = bacc.Bacc("TRN2", target_bir_lowering=False, debug=False,
                   num_devices=NCORES)

    # pbox planes: [128, IPC*16*4] each, r-major: [p, ((i*16+t)*4+c)]
    pbhi_d = nc.dram_tensor("pbhi", [128, IPC * 16 * 4], bf16,
                            kind="ExternalInput").ap()
    pblo1_d = nc.dram_tensor("pblo1", [128, IPC * 16 * 4], bf16,
                             kind="ExternalInput").ap()
    pblo2_d = nc.dram_tensor("pblo2", [128, IPC * 16 * 4], bf16,
                             kind="ExternalInput").ap()
    # gtb5[i, c5, 200]: gx1, gy1, gx2, gy2, garea (f32)
    gtb5_d = nc.dram_tensor("gtb5", [IPC, 5, M], f32, kind="ExternalInput").ap()
    thr_d = nc.dram_tensor("thr", [CH, 1], f32, kind="ExternalInput").ap()
    siota_d = nc.dram_tensor("siota", [K], f16, kind="ExternalInput").ap()
    cumind_d = nc.dram_tensor("cumind", [IPC, RPAD], f16,
                              kind="ExternalInput").ap()
    tp_d = nc.dram_tensor("tp", [CH, 1], f32, kind="ExternalOutput").ap()

    cbox_dram = nc.dram_tensor("cbox_s", [NCH, 128, IPC, 4], f32).ap()
    # ciou_s[q, i, s, c]: phase-B partition p = i*8 + s
    ciou_dram = nc.dram_tensor("ciou_s", [NQ, IPC, 8, M], f32).ap()

    with tile.TileContext(nc) as tc, ExitStack() as ctx:
        cpool = ctx.enter_context(tc.tile_pool(name="const", bufs=1))
        siota_b = cpool.tile([128, K], f16)
        nc.sync.dma_start(siota_b[:], siota_d.unsqueeze(0).to_broadcast([128, K]))
        thr_sb = cpool.tile([CH, 1], f32)
        nc.sync.dma_start(thr_sb[:], thr_d[:])
        pbhi = cpool.tile([128, IPC * 16 * 4], bf16)
        pblo1 = cpool.tile([128, IPC * 16 * 4], bf16)
        pblo2 = cpool.tile([128, IPC * 16 * 4], bf16)
        nc.sync.dma_start(pbhi[:], pbhi_d[:, :])
        nc.sync.dma_start(pblo1[:], pblo1_d[:, :])
        nc.sync.dma_start(pblo2[:], pblo2_d[:, :])
        # cumind as [128, IPC*16] columns: p, (i, t); f32 for is_equal scalar
        cic = cpool.tile([128, IPC * NRT], f16)
        nc.sync.dma_start(
            cic[:],
            cumind_d.rearrange("i (t p) -> p (i t)", p=128),
        )
        cic32 = cpool.tile([128, IPC * NRT], f32)
        nc.scalar.copy(out=cic32[:], in_=cic[:])
        # gt broadcast [128=(i*8+s), 5*200]
        gtb = cpool.tile([128, 5 * M], f32)
        nc.sync.dma_start(
            gtb[:],
            gtb5_d.unsqueeze(1).to_broadcast([IPC, 8, 5, M]),
        )
        # candidate boxes per q: [128=(s,i), NQ*4]
        cball = cpool.tile([128, NQ * 4], f32)

        # ---------------- phase A: gather candidate boxes ------------------
        with (
            tc.tile_pool(name="ohp", bufs=2) as ohpool,
            tc.tile_pool(name="gps", bufs=1, space="PSUM") as gpsum,
            tc.tile_pool(name="cbs", bufs=1) as cbpool,
        ):
            cbsb = [cbpool.tile([128, IPC * 4], f32, tag="cbsb%d" % ch,
                                name="cbsb%d" % ch) for ch in range(NCH)]
            for i in range(IPC):
                ohs = []
                for t in range(NRT):
                    oh = ohpool.tile([128, K], bf16, tag="oh%d" % t,
                                     name="oh%d" % t)
                    nc.vector.tensor_scalar(
                        out=oh[:], in0=siota_b[:],
                        scalar1=cic32[:, i * NRT + t : i * NRT + t + 1],
                        scalar2=None, op0=OP.is_equal,
                    )
                    ohs.append(oh)
                for ci, (s0, srows) in enumerate(chunks):
                    psh = gpsum.tile([srows, 4], f32, tag="psh", name="psh")
                    psl = gpsum.tile([srows, 4], f32, tag="psl", name="psl")
                    for t in range(NRT):
                        rhs_h = pbhi[:, (i * 16 + t) * 4 : (i * 16 + t + 1) * 4]
                        rhs_l1 = pblo1[:, (i * 16 + t) * 4 : (i * 16 + t + 1) * 4]
                        rhs_l2 = pblo2[:, (i * 16 + t) * 4 : (i * 16 + t + 1) * 4]
                        lhs = ohs[t][:, s0 : s0 + srows]
                        nc.tensor.matmul(out=psh[:], lhsT=lhs, rhs=rhs_h,
                                         start=(t == 0), stop=(t == NRT - 1))
                        nc.tensor.matmul(out=psl[:], lhsT=lhs, rhs=rhs_l1,
                                         start=(t == 0), stop=False)
                        nc.tensor.matmul(out=psl[:], lhsT=lhs, rhs=rhs_l2,
                                         start=False, stop=(t == NRT - 1))
                    hsb = ohpool.tile([128, 4], f32, tag="hsb", name="hsb")
                    nc.scalar.copy(out=hsb[:srows], in_=psh[:])
                    nc.vector.tensor_tensor(
                        out=cbsb[ci][:srows, i * 4 : (i + 1) * 4],
                        in0=hsb[:srows], in1=psl[:], op=OP.add,
                    )
            for ci, (s0, srows) in enumerate(chunks):
                nc.sync.dma_start(cbox_dram[ci, :srows], cbsb[ci][:srows])

        # cball[p=(i*8+s), (q,c)] <- cbox_dram[chunk, slot=q*8+s, i, c]
        for ci in range(NCH):
            q0 = ci * 16
            qn = min(16, NQ - q0)
            if qn <= 0:
                break
            for i in range(IPC):
                src = cbox_dram[ci, :, i, :].rearrange("(q s) c -> s q c", s=8)
                nc.sync.dma_start(
                    cball[i * 8 : (i + 1) * 8, q0 * 4 : (q0 + qn) * 4],
                    src[:, :qn, :],
                )

        # ---------------- phase B: exact iou per k-group -------------------
        with tc.tile_pool(name="rb", bufs=2) as rpool:
            car_all = cpool.tile([128, NQ], f32)
            cw = rpool.tile([128, NQ], f32, tag="cw", name="cw")
            chh = rpool.tile([128, NQ], f32, tag="chh", name="chh")
            nc.vector.tensor_tensor(
                out=cw[:], in0=cball[:, 2::4], in1=cball[:, 0::4],
                op=OP.subtract,
            )
            nc.vector.tensor_tensor(
                out=chh[:], in0=cball[:, 3::4], in1=cball[:, 1::4],
                op=OP.subtract,
            )
            nc.vector.tensor_tensor(
                out=car_all[:], in0=cw[:], in1=chh[:], op=OP.mult
            )
            gx1 = gtb[:, 0 * M : 1 * M]
            gy1 = gtb[:, 1 * M : 2 * M]
            gx2 = gtb[:, 2 * M : 3 * M]
            gy2 = gtb[:, 3 * M : 4 * M]
            gab = gtb[:, 4 * M : 5 * M]
            for q in range(NQ):
                cx1 = cball[:, q * 4 + 0 : q * 4 + 1]
                cy1 = cball[:, q * 4 + 1 : q * 4 + 2]
                cx2 = cball[:, q * 4 + 2 : q * 4 + 3]
                cy2 = cball[:, q * 4 + 3 : q * 4 + 4]
                car = car_all[:, q : q + 1]
                m2x = rpool.tile([128, M], f32, tag="m2x", name="m2x")
                m2y = rpool.tile([128, M], f32, tag="m2y", name="m2y")
                wxn = rpool.tile([128, M], f32, tag="wxn", name="wxn")
                wyn = rpool.tile([128, M], f32, tag="wyn", name="wyn")
                rwx = rpool.tile([128, M], f32, tag="rwx", name="rwx")
                rwy = rpool.tile([128, M], f32, tag="rwy", name="rwy")
                inter = rpool.tile([128, M], f32, tag="inter", name="inter")
                u = rpool.tile([128, M], f32, tag="u", name="u")
                rec = rpool.tile([128, M], f32, tag="rec", name="rec")
                iou = rpool.tile([128, M], f32, tag="iou", name="iou")
                nc.vector.tensor_scalar(
                    out=m2x[:], in0=gx2, scalar1=cx2, scalar2=None, op0=OP.min,
                )
                nc.vector.scalar_tensor_tensor(
                    out=wxn[:], in0=gx1, scalar=cx1, in1=m2x[:],
                    op0=OP.max, op1=OP.subtract,
                )
                nc.scalar.activation(out=rwx[:], in_=wxn[:], func=AF.Relu,
                                     scale=-1.0)
                nc.vector.tensor_scalar(
                    out=m2y[:], in0=gy2, scalar1=cy2, scalar2=None, op0=OP.min,
                )
                nc.vector.scalar_tensor_tensor(
                    out=wyn[:], in0=gy1, scalar=cy1, in1=m2y[:],
                    op0=OP.max, op1=OP.subtract,
                )
                nc.scalar.activation(out=rwy[:], in_=wyn[:], func=AF.Relu,
                                     scale=-1.0)
                nc.vector.tensor_tensor(
                    out=inter[:], in0=rwx[:], in1=rwy[:], op=OP.mult
                )
                # u = (gab + car) - inter
                nc.vector.scalar_tensor_tensor(
                    out=u[:], in0=gab, scalar=car, in1=inter[:],
                    op0=OP.add, op1=OP.subtract,
                )
                nc.vector.reciprocal(out=rec[:], in_=u[:])
                nc.vector.tensor_tensor(
                    out=iou[:], in0=inter[:], in1=rec[:], op=OP.mult
                )
                nc.sync.dma_start(ciou_dram[q], iou[:])

        # ---------------- phase C: greedy scan ----------------------------
        with (
            tc.tile_pool(name="rowg", bufs=3) as rowpool,
            tc.tile_pool(name="scw", bufs=2) as swpool,
            tc.tile_pool(name="fst", bufs=2) as fpool,
        ):
            f = fpool.tile([CH, M], f32, tag="f", name="f0")
            nc.vector.memset(f[:], 0.0)
            for q in range(NQ):
                rowg = rowpool.tile([CH, 8 * M], f32, tag="rowg", name="rowg")
                nc.sync.dma_start(
                    rowg[:],
                    ciou_dram[q].unsqueeze(1).to_broadcast([IPC, NT, 8, M]),
                )
                for s in range(8):
                    row = rowg[:, s * M : (s + 1) * M]
                    masked = swpool.tile([CH, M], f32, tag="masked",
                                         name="masked")
                    vt = swpool.tile([CH, 1], f32, tag="vt", name="vt")
                    # (tensor_tensor_reduce crashes this HW runtime; use
                    # the unfused TT + reduce + tiny-ts sequence instead)
                    nc.vector.tensor_tensor(
                        out=masked[:], in0=row, in1=f[:], op=OP.subtract
                    )
                    nc.vector.reduce_max(out=vt[:], in_=masked[:], axis=AX.X)
                    nc.vector.tensor_scalar(
                        out=vt[:], in0=vt[:], scalar1=thr_sb[:, 0:1],
                        scalar2=None, op0=OP.max,
                    )
                    fn = fpool.tile([CH, M], f32, tag="f", name="fn")
                    nc.vector.scalar_tensor_tensor(
                        out=fn[:], in0=masked[:], scalar=vt[:, 0:1], in1=f[:],
                        op0=OP.is_ge, op1=OP.max,
                    )
                    f = fn
            tpo = swpool.tile([CH, 1], f32, tag="tpo", name="tpo")
            nc.vector.reduce_sum(out=tpo[:], in_=f[:], axis=AX.X)
            nc.sync.dma_start(tp_d[:, :], tpo[:])

    nc.compile()
    return nc


def _get_p1():
    if "p1" not in _CACHE:
        _CACHE["p1"] = _build_p1()
    return _CACHE["p1"]


def _get_p2(K):
    key = "p2_%d" % K
    if key not in _CACHE:
        _CACHE[key] = _build_p2(K)
    return _CACHE[key]


def _prep_core(pred, gt):
    """Host-side input prep for one core's 16-image shard."""
    f16 = np.float16
    # --- p1 tensors ---
    pbh = np.full((IPC, 5, RPAD), 30000.0, f16)
    pbh[:, 0, :N] = pred[:, :, 0].astype(f16)
    pbh[:, 1, :N] = pred[:, :, 1].astype(f16)
    pbh[:, 2, :N] = pred[:, :, 2].astype(f16)
    pbh[:, 3, :N] = pred[:, :, 3].astype(f16)
    parea = (pred[:, :, 2] - pred[:, :, 0]) * (pred[:, :, 3] - pred[:, :, 1])
    pbh[:, 4, :N] = parea.astype(f16)

    gw = gt[:, :, 2] - gt[:, :, 0]
    gh = gt[:, :, 3] - gt[:, :, 1]
    garea = gw * gh
    eps = 1.8 * (gw + gh) + 44.0   # covers fp16 rounding incl. gp plane
    ga_margin = garea - eps              # cmp: 3*inter - ga_margin >= parea
    colsel = np.zeros((128, IPC * IPC), np.float32)
    for i in range(IPC):
        colsel[:, i * IPC + i] = 1.0
    gsc = np.zeros((128, IPC * 2 * 5), np.float32)
    for i in range(IPC):
        for tix, (off, rows) in enumerate(GT_TILES):
            sc = (i * 2 + tix) * 5
            gsc[:rows, sc + 0] = gt[i, off : off + rows, 0]
            gsc[:rows, sc + 1] = gt[i, off : off + rows, 1]
            gsc[:rows, sc + 2] = gt[i, off : off + rows, 2]
            gsc[:rows, sc + 3] = gt[i, off : off + rows, 3]
            gsc[:rows, sc + 4] = ga_margin[i, off : off + rows]
            if rows < 128:
                gsc[rows:, sc + 0 : sc + 4] = 30000.0
                gsc[rows:, sc + 4] = 1e9

    # --- p2 tensors ---
    # r-major pred planes, hi/lo1/lo2 bf16 split
    pb_r = np.zeros((128, IPC * 16 * 4), np.float32)
    for t in range(NRT):
        r0 = t * 128
        rows = min(128, N - r0)
        # pb_r[p, (i*16+t)*4 + c] = pred[i, r0+p, c]
        blk = pred[:, r0 : r0 + rows, :]          # [IPC, rows, 4]
        idx = (np.arange(IPC) * 16 + t)[:, None] * 4 + np.arange(4)[None, :]
        pb_r[:rows, idx.reshape(-1)] = blk.transpose(1, 0, 2).reshape(rows, -1)
    hi = (np.floor(pb_r / 4.0) * 4.0).astype(np.float32)
    lo = pb_r - hi

    # numpy lacks bfloat16; emulate rounding via float32 bit manipulation
    def to_bf16_f32(x):
        xb = x.astype(np.float32).view(np.uint32)
        xb = (xb + 0x8000) & 0xFFFF0000
        return xb.view(np.float32)
    lo1f = to_bf16_f32(lo)
    lo2f = lo - lo1f
    pbhi = hi
    pblo1 = lo1f
    pblo2 = lo2f

    gtb5 = np.zeros((IPC, 5, M), np.float32)
    gtb5[:, 0] = gt[:, :, 0]
    gtb5[:, 1] = gt[:, :, 1]
    gtb5[:, 2] = gt[:, :, 2]
    gtb5[:, 3] = gt[:, :, 3]
    gtb5[:, 4] = garea

    thr = np.repeat(THRESHOLDS[None, :], IPC, axis=0).reshape(CH, 1)
    return {
        "pbh": pbh,
        "gsc": gsc,
        "colsel": colsel,
        "pbhi": pbhi,
        "pblo1": pblo1,
        "pblo2": pblo2,
        "gtb5": gtb5,
        "thr": np.ascontiguousarray(thr, np.float32),
    }


def _np_to_bf16_arrays(maps):
    """Convert the bf16-destined fp32 arrays to uint16 bf16 bit patterns
    if the runtime wants raw bf16; run_bass_kernel_spmd handles fp32->bf16
    via ml_dtypes if available."""
    try:
        import ml_dtypes
        for m in maps:
            for k in ("pbhi", "pblo1", "pblo2", "colsel"):
                if k in m:
                    m[k] = m[k].astype(ml_dtypes.bfloat16)
    except ImportError:
        pass
    return maps


def _pick_k(cmax):
    want = min(cmax + 8, N)
    return next(k for k in K_BUCKETS if k >= want)


def kernel(pred_boxes, gt_boxes):
    from concourse.bass_utils import run_bass_kernel_spmd

    pred_boxes = np.ascontiguousarray(pred_boxes, np.float32)
    gt_boxes = np.ascontiguousarray(gt_boxes, np.float32)

    preps = []
    p1_maps = []
    for c in range(NCORES):
        pr = _prep_core(pred_boxes[c * IPC : (c + 1) * IPC],
                        gt_boxes[c * IPC : (c + 1) * IPC])
        preps.append(pr)
        p1_maps.append({"pbh": pr["pbh"], "gsc": pr["gsc"],
                        "colsel": pr["colsel"]})
    _np_to_bf16_arrays(p1_maps)

    res1 = run_bass_kernel_spmd(_get_p1(), p1_maps, list(range(NCORES)))
    cmax = max(int(np.float32(r["cc"].max())) for r in res1.results)
    K = _pick_k(cmax)

    siota = np.arange(1, K + 1, dtype=np.float16)
    in_maps = []
    for c in range(NCORES):
        pr = preps[c]
        in_maps.append({
            "pbhi": pr["pbhi"],
            "pblo1": pr["pblo1"],
            "pblo2": pr["pblo2"],
            "gtb5": pr["gtb5"],
            "thr": pr["thr"],
            "siota": siota,
            "cumind": res1.results[c]["cumind"],
        })
    _np_to_bf16_arrays(in_maps)
    res2 = run_bass_kernel_spmd(_get_p2(K), in_maps, list(range(NCORES)))
    tp = np.concatenate([r["tp"].reshape(-1) for r in res2.results])
    tp = tp.astype(np.float32)
    prec = tp / (np.float32(N + M) - tp)
    per_img = prec.reshape(B, NT).mean(axis=1, dtype=np.float32)
    return np.float32(per_img.mean(dtype=np.float32))
